# revision 30
# baseline (speedup 1.0000x reference)
# Trainium2 Bass kernel for FJSP actor head (gnn_message_passing).
#
# Math (per batch b):
#   job_emb = ops_emb[b, next_op[b], :]                  [50, 128]  (gather)
#   u_j = job_emb @ W1[:128]   v_m = ma_emb[b] @ W1[128:]
#   h1[j,m] = relu(u_j + v_m + b1)            -> 2000 pairs + 1 noop (dummy)
#   h2 = relu(h1 @ W2 + b2);  logit = h2 @ W3 + b3
#
# Device strategy (pure data parallel over batch, 32 batches/core):
#   * The pairwise broadcast u_j + v_m + b1 is ONE matmul per batch:
#     lhsT = JV (rows: 50 u's, 40 v's, dummy@W1, b1) [106, 128],
#     rhs = S, a constant 0/1 selection matrix built on host.
#     Column 0 of S selects {dummy@W1, b1} = the noop logit pipeline.
#   * Gather rows are cast to bf16 by a gpsimd DMA and transposed by the
#     DMA xbar (no PE/ACT/DVE involvement).
#   * Per batch, stage-major: 4x S-matmul -> one full-width relu1 ->
#     4x W2-matmul -> relu2(+b2) -> W3-matmul col-tiled into psum
#     partitions {0,32,64,96} -> one wide copy -> one DMA per row.
#   * All matmuls bf16 (fp32 psum accumulate); b3 added on host.

import numpy as np
from contextlib import ExitStack

import concourse.bass as bass
import concourse.mybir as mybir
import concourse.tile as tile
from concourse import bacc
from concourse.bass_utils import run_bass_kernel_spmd

BS, N_OPS, N_JOBS, N_MA, E, H = 256, 2000, 50, 40, 128, 128
NCORES = 8
BPC = BS // NCORES            # 32 batches per core
NPAIR = N_JOBS * N_MA + 1     # 2001 logits per batch (col 0 = noop)
NPAD = 2048                   # padded logit row (cols 2001:2048 are junk)
PB = 64                       # gather rows reserved per batch (50 real + 14 pad)
NCHUNK = BPC * PB // 128      # 16 gather chunks of 128 rows
# JV partition layout (K = 106)
R_V0 = 64                     # v_m rows 64..103  (u_j rows at 0..49)
R_ZD = 104                    # dummy @ W1 row
R_B1 = 105                    # b1 row
KJV = 106
NCH = 4                       # 512-col chunks per logit row

f32 = mybir.dt.float32
bf16 = mybir.dt.bfloat16

Relu = mybir.ActivationFunctionType.Relu


def _build_smat() -> np.ndarray:
    S = np.zeros((KJV, NPAD), np.float32)
    S[R_B1, :NPAIR] = 1.0
    S[R_ZD, 0] = 1.0
    for j in range(N_JOBS):
        S[j, 1 + j * N_MA: 1 + (j + 1) * N_MA] = 1.0
    for m in range(N_MA):
        S[R_V0 + m, 1 + m: NPAIR: N_MA] = 1.0
    return S


def _build_module() -> bass.Bass:
    nc = bacc.Bacc("TRN2", target_bir_lowering=False, debug=False)
    ops = nc.dram_tensor("ops", [BPC * N_OPS, E], f32, kind="ExternalInput")
    ma = nc.dram_tensor("ma", [BPC * N_MA, E], f32, kind="ExternalInput")
    idx = nc.dram_tensor("idx", [128, NCHUNK], mybir.dt.int32, kind="ExternalInput")
    smat = nc.dram_tensor("smat", [KJV, NPAD], bf16, kind="ExternalInput")
    w1 = nc.dram_tensor("w1", [2 * E, H], bf16, kind="ExternalInput")
    w2 = nc.dram_tensor("w2", [H, H], bf16, kind="ExternalInput")
    w3 = nc.dram_tensor("w3", [H, 1], bf16, kind="ExternalInput")
    b1v = nc.dram_tensor("b1v", [H], bf16, kind="ExternalInput")
    b2v = nc.dram_tensor("b2v", [H], f32, kind="ExternalInput")
    dvec = nc.dram_tensor("dvec", [2 * E], f32, kind="ExternalInput")
    out = nc.dram_tensor("out", [BPC, NPAD], f32, kind="ExternalOutput")

    with tile.TileContext(nc) as tc, ExitStack() as ctx:
        singles = ctx.enter_context(tc.tile_pool(name="singles", bufs=1))

        # ---- input loads, ordered so the gather chain starts ASAP ----
        idx_s = singles.tile([128, NCHUNK], mybir.dt.int32)
        nc.sync.dma_start(out=idx_s[:], in_=idx[:])

        wj_s = singles.tile([128, H], bf16)
        nc.sync.dma_start(out=wj_s[:], in_=w1[0:E, :])
        wm_s = singles.tile([128, H], bf16)
        nc.sync.dma_start(out=wm_s[:], in_=w1[E:2 * E, :])
        w2_s = singles.tile([128, H], bf16)
        nc.sync.dma_start(out=w2_s[:], in_=w2[:])
        w3_s = singles.tile([128, 1], bf16)
        nc.sync.dma_start(out=w3_s[:], in_=w3[:])
        smat_s = singles.tile([KJV, NPAD], bf16)
        nc.sync.dma_start(out=smat_s[:], in_=smat[:])

        dcols = singles.tile([128, 2], bf16)
        nc.gpsimd.dma_start(out=dcols[:, 0:1],
                            in_=dvec[0:E].rearrange("(p o) -> p o", o=1))
        nc.gpsimd.dma_start(out=dcols[:, 1:2],
                            in_=dvec[E:2 * E].rearrange("(p o) -> p o", o=1))

        # all 16 indirect gathers first on the gpsimd queue, then bf16 casts
        grows_pool = ctx.enter_context(tc.tile_pool(name="growsp", bufs=NCHUNK))
        grows = [grows_pool.tile([128, E], f32, tag="grows", name=f"grows{c}")
                 for c in range(NCHUNK)]
        mrows_pool = ctx.enter_context(tc.tile_pool(name="mrowsp", bufs=NCHUNK))
        mrows = [mrows_pool.tile([128, E], f32, tag="mrows", name=f"mrows{c}")
                 for c in range(NCHUNK)]
        for c in range(NCHUNK):
            nc.gpsimd.indirect_dma_start(
                out=grows[c][:], out_offset=None, in_=ops[:],
                in_offset=bass.IndirectOffsetOnAxis(ap=idx_s[:, c:c + 1], axis=0),
            )
            nc.sync.dma_start(out=mrows[c][0:2 * N_MA, :],
                              in_=ma[2 * c * N_MA:(2 * c + 2) * N_MA, :])

        # small partition-strided loads on the scalar HWDGE ring
        b2_s = singles.tile([128, 1], f32)
        nc.scalar.dma_start(out=b2_s[:], in_=b2v[:].rearrange("(p o) -> p o", o=1))
        ident = singles.tile([128, 128], f32)
        from concourse.masks import make_identity
        make_identity(nc, ident[:])

        # zdb1: row 0 = dummy@W1 (filled below), row 1 = b1
        zdb1 = singles.tile([2, 128], bf16)
        nc.scalar.dma_start(out=zdb1[1:2, :],
                            in_=b1v[:].rearrange("(o e) -> o e", o=1))

        # xbar transposes (bf16, 16-bit path), off the compute engines
        jt_pool = ctx.enter_context(tc.tile_pool(name="jt", bufs=4))
        mt_pool = ctx.enter_context(tc.tile_pool(name="mt", bufs=4))

        # per-chunk JV tiles (2 batches side by side).  Rows 50:64 hold junk
        # projections of the finite pad gather rows; smat rows there are 0.
        jv_pool = ctx.enter_context(tc.tile_pool(name="jvp", bufs=NCHUNK))
        jvp = [jv_pool.tile([128, 2 * 128], bf16, tag="jv", name=f"jv{c}")
               for c in range(NCHUNK)]

        setup_ps = ctx.enter_context(tc.tile_pool(name="sps", bufs=1, space="PSUM"))
        h1_ps = ctx.enter_context(tc.tile_pool(name="h1ps", bufs=3, space="PSUM"))
        h2_ps = ctx.enter_context(tc.tile_pool(name="h2ps", bufs=2, space="PSUM"))
        lg_ps = ctx.enter_context(tc.tile_pool(name="lgps", bufs=2, space="PSUM"))
        a_pool = ctx.enter_context(tc.tile_pool(name="ap", bufs=4))
        h2s_pool = ctx.enter_context(tc.tile_pool(name="h2s", bufs=4))
        st_pool = ctx.enter_context(tc.tile_pool(name="st", bufs=3))

        # PE warm-up during the initial DMA window: junk matmuls (HAM)
        warm = singles.tile([128, 512], bf16)
        nc.vector.memset(warm[:].bitcast(mybir.dt.uint16), 0)
        for _ in range(8):
            wp = setup_ps.tile([128, 512], f32, tag="sps", name="wp")
            nc.tensor.matmul(out=wp[:], lhsT=warm[:, 0:128], rhs=warm[:],
                             start=True, stop=True)

        # dummy @ W1 (once): [1,128] = dcols[:,0].T @ Wj + dcols[:,1].T @ Wm
        pd = setup_ps.tile([1, 128], f32, tag="sps", name="pd")
        nc.tensor.matmul(out=pd[:], lhsT=dcols[:, 0:1], rhs=wj_s[:],
                         start=True, stop=False)
        nc.tensor.matmul(out=pd[:], lhsT=dcols[:, 1:2], rhs=wm_s[:],
                         start=False, stop=True)
        nc.vector.tensor_copy(out=zdb1[0:1, :], in_=pd[:])

        for c in range(NCHUNK):
            tpj = setup_ps.tile([128, 128], f32, tag="sps")
            nc.tensor.transpose(out=tpj[:], in_=grows[c][:], identity=ident[:])
            jTc = jt_pool.tile([128, 128], bf16, tag="jt")
            nc.scalar.copy(out=jTc[:], in_=tpj[:])
            tpm = setup_ps.tile([128, 128], f32, tag="sps")
            nc.tensor.transpose(out=tpm[:], in_=mrows[c][:], identity=ident[:])
            mTc = mt_pool.tile([128, 128], bf16, tag="mt")
            nc.vector.tensor_copy(out=mTc[:], in_=tpm[:])

            bb = (2 * c, 2 * c + 1)
            # ---- projections for both batches into jvp[c] ----
            pj = setup_ps.tile([128, 2 * 128], f32, tag="sps", name=f"pj{c}")
            for sub in range(2):
                nc.tensor.matmul(out=pj[0:PB, 128 * sub:128 * (sub + 1)],
                                 lhsT=jTc[:, sub * PB: (sub + 1) * PB],
                                 rhs=wj_s[:], start=True, stop=True)
                nc.tensor.matmul(out=pj[R_V0:R_V0 + N_MA, 128 * sub:128 * (sub + 1)],
                                 lhsT=mTc[:, sub * N_MA: (sub + 1) * N_MA],
                                 rhs=wm_s[:], start=True, stop=True)
            nc.scalar.copy(out=jvp[c][0:PB, :], in_=pj[0:PB, :])
            nc.vector.tensor_copy(out=jvp[c][R_V0:R_V0 + N_MA, :],
                                  in_=pj[R_V0:R_V0 + N_MA, :])
            nc.scalar.dma_start(
                out=jvp[c][R_ZD:R_B1 + 1, :].rearrange("p (r e) -> p r e", r=2),
                in_=zdb1[:].rearrange("p (o e) -> p o e", o=1).to_broadcast([2, 2, 128]))

            # ---- main pipelines, two batches interleaved ----
            lgps = [lg_ps.tile([128, 512], f32, tag="lg", name=f"lg{b}")
                    for b in bb]
            for ci in range(NCH):
                for sub in range(2):
                    b = bb[sub]
                    h1p = h1_ps.tile([128, 512], f32, tag="h1p")
                    nc.tensor.matmul(
                        out=h1p[:],
                        lhsT=jvp[c][0:KJV, 128 * sub:128 * (sub + 1)],
                        rhs=smat_s[:, 512 * ci:512 * (ci + 1)],
                        start=True, stop=True)
                    A = a_pool.tile([128, 512], bf16, tag="A")
                    if ci == 3 and (c + sub) % 2 == 0:
                        nc.scalar.activation(out=A[:], in_=h1p[:], func=Relu)
                    else:
                        nc.vector.tensor_scalar_max(out=A[:], in0=h1p[:],
                                                    scalar1=0.0)
                    h2p = h2_ps.tile([128, 512], f32, tag="h2p")
                    nc.tensor.matmul(out=h2p[:], lhsT=w2_s[:], rhs=A[:],
                                     start=True, stop=True)
                    H2 = h2s_pool.tile([128, 512], bf16, tag="H2")
                    nc.scalar.activation(out=H2[:], in_=h2p[:],
                                         func=Relu, bias=b2_s[:, 0:1])
                    # logits chunk -> psum partition 32*ci
                    nc.tensor.matmul(out=lgps[sub][32 * ci:32 * ci + 1, :],
                                     lhsT=w3_s[:], rhs=H2[:],
                                     start=True, stop=True,
                                     tile_position=(0, 32 * ci))
            for sub in range(2):
                b = bb[sub]
                # one wide copy (only rows 0/32/64/96 carry data)
                stg = st_pool.tile([128, 512], f32, tag="st")
                if b % 2 == 0:
                    nc.scalar.copy(out=stg[0:97, :], in_=lgps[sub][0:97, :])
                else:
                    nc.vector.tensor_copy(out=stg[0:97, :], in_=lgps[sub][0:97, :])
                stg4 = stg[:].rearrange("(a b) f -> a b f", b=32)[:, 0:1, :]
                nc.sync.dma_start(
                    out=out[b:b + 1, :].rearrange("o (a f) -> o a f", a=4),
                    in_=stg4)

    nc.finalize()
    return nc


_CACHE: dict = {}


def _get_module() -> bass.Bass:
    if "nc" not in _CACHE:
        _CACHE["nc"] = _build_module()
    return _CACHE["nc"]


def _make_in_maps(inputs):
    import ml_dtypes
    bf = ml_dtypes.bfloat16

    ops_emb = np.ascontiguousarray(np.asarray(inputs["ops_emb"], dtype=np.float32))
    ma_emb = np.ascontiguousarray(np.asarray(inputs["ma_emb"], dtype=np.float32))
    next_op = np.asarray(inputs["next_op"])
    dummy = np.asarray(inputs["dummy"], dtype=np.float32)
    W1 = np.ascontiguousarray(np.asarray(inputs["W1"], dtype=np.float32).astype(bf))
    b1 = np.asarray(inputs["b1"], dtype=np.float32).astype(bf)
    W2 = np.ascontiguousarray(np.asarray(inputs["W2"], dtype=np.float32).astype(bf))
    b2 = np.ascontiguousarray(np.asarray(inputs["b2"], dtype=np.float32))
    W3 = np.ascontiguousarray(np.asarray(inputs["W3"], dtype=np.float32).astype(bf))
    smat = _build_smat().astype(bf)

    in_maps = []
    for core in range(NCORES):
        bsl = slice(core * BPC, (core + 1) * BPC)
        no = np.asarray(next_op[bsl], dtype=np.int64)          # [BPC, 50]
        gidx = np.zeros((BPC, PB), np.int64)
        gidx[:, :N_JOBS] = no + (np.arange(BPC, dtype=np.int64)[:, None] * N_OPS)
        idx2d = np.ascontiguousarray(
            gidx.reshape(NCHUNK, 128).T.astype(np.int32))      # [128, NCHUNK]
        in_maps.append({
            "ops": ops_emb[bsl].reshape(BPC * N_OPS, E),
            "ma": ma_emb[bsl].reshape(BPC * N_MA, E),
            "idx": idx2d,
            "smat": smat,
            "w1": W1, "w2": W2, "w3": W3,
            "b1v": b1, "b2v": b2, "dvec": dummy,
        })
    return in_maps


def _run(inputs, trace=False, **kw):
    action_mask = np.asarray(inputs["action_mask"])
    b3 = np.asarray(inputs["b3"], dtype=np.float32)
    nc = _get_module()
    in_maps = _make_in_maps(inputs)
    res = run_bass_kernel_spmd(nc, in_maps, core_ids=list(range(NCORES)),
                               trace=trace, **kw)
    logits = np.concatenate([r["out"][:, :NPAIR] for r in res.results], axis=0)
    logits = (logits + b3.reshape(-1)[0]).astype(np.float32)
    return (logits, action_mask), res


def kernel(**inputs):
    out, _ = _run(inputs)
    return out


# revision 31
# speedup vs baseline: 1.2018x; 1.2018x over previous
# Trainium2 Bass kernel for FJSP actor head (gnn_message_passing).
#
# Math (per batch b):
#   job_emb = ops_emb[b, next_op[b], :]                  [50, 128]  (gather)
#   u_j = job_emb @ W1[:128]   v_m = ma_emb[b] @ W1[128:]
#   h1[j,m] = relu(u_j + v_m + b1)            -> 2000 pairs + 1 noop (dummy)
#   h2 = relu(h1 @ W2 + b2);  logit = h2 @ W3 + b3
#
# Device strategy (pure data parallel over batch, 32 batches/core):
#   * The pairwise broadcast u_j + v_m + b1 is ONE matmul per batch:
#     lhsT = JV (rows: 50 u's, 40 v's, dummy@W1, b1) [106, 128],
#     rhs = S, a constant 0/1 selection matrix built on host.
#     Column 0 of S selects {dummy@W1, b1} = the noop logit pipeline.
#   * Gather rows are cast to bf16 by a gpsimd DMA and transposed by the
#     DMA xbar (no PE/ACT/DVE involvement).
#   * Per batch, stage-major: 4x S-matmul -> one full-width relu1 ->
#     4x W2-matmul -> relu2(+b2) -> W3-matmul col-tiled into psum
#     partitions {0,32,64,96} -> one wide copy -> one DMA per row.
#   * All matmuls bf16 (fp32 psum accumulate); b3 added on host.

import numpy as np
from contextlib import ExitStack

import concourse.bass as bass
import concourse.mybir as mybir
import concourse.tile as tile
from concourse import bacc
from concourse.bass_utils import run_bass_kernel_spmd

BS, N_OPS, N_JOBS, N_MA, E, H = 256, 2000, 50, 40, 128, 128
NCORES = 8
BPC = BS // NCORES            # 32 batches per core
NPAIR = N_JOBS * N_MA + 1     # 2001 logits per batch (col 0 = noop)
NPAD = 2048                   # padded logit row (cols 2001:2048 are junk)
PB = 64                       # gather rows reserved per batch (50 real + 14 pad)
NCHUNK = BPC * PB // 128      # 16 gather chunks of 128 rows
# JV partition layout (K = 106)
R_V0 = 64                     # v_m rows 64..103  (u_j rows at 0..49)
R_ZD = 104                    # dummy @ W1 row
R_B1 = 105                    # b1 row
KJV = 106
NCH = 4                       # 512-col chunks per logit row

f32 = mybir.dt.float32
bf16 = mybir.dt.bfloat16

Relu = mybir.ActivationFunctionType.Relu


def _build_smat() -> np.ndarray:
    S = np.zeros((KJV, NPAD), np.float32)
    S[R_B1, :NPAIR] = 1.0
    S[R_ZD, 0] = 1.0
    for j in range(N_JOBS):
        S[j, 1 + j * N_MA: 1 + (j + 1) * N_MA] = 1.0
    for m in range(N_MA):
        S[R_V0 + m, 1 + m: NPAIR: N_MA] = 1.0
    return S


def _build_module() -> bass.Bass:
    nc = bacc.Bacc("TRN2", target_bir_lowering=False, debug=False)
    ops = nc.dram_tensor("ops", [BPC * N_OPS, E], f32, kind="ExternalInput")
    ma = nc.dram_tensor("ma", [BPC * N_MA, E], f32, kind="ExternalInput")
    idx = nc.dram_tensor("idx", [128, NCHUNK], mybir.dt.int32, kind="ExternalInput")
    smat = nc.dram_tensor("smat", [KJV, NPAD], bf16, kind="ExternalInput")
    w1 = nc.dram_tensor("w1", [2 * E, H], bf16, kind="ExternalInput")
    w2 = nc.dram_tensor("w2", [H, H], bf16, kind="ExternalInput")
    w3 = nc.dram_tensor("w3", [H, 1], bf16, kind="ExternalInput")
    b1v = nc.dram_tensor("b1v", [H], bf16, kind="ExternalInput")
    b2v = nc.dram_tensor("b2v", [H], f32, kind="ExternalInput")
    dvec = nc.dram_tensor("dvec", [2 * E], f32, kind="ExternalInput")
    out = nc.dram_tensor("out", [BPC, NPAD], f32, kind="ExternalOutput")

    with tile.TileContext(nc) as tc, ExitStack() as ctx:
        singles = ctx.enter_context(tc.tile_pool(name="singles", bufs=1))

        # ---- input loads, ordered so the gather chain starts ASAP ----
        idx_s = singles.tile([128, NCHUNK], mybir.dt.int32)
        nc.sync.dma_start(out=idx_s[:], in_=idx[:])

        wj_s = singles.tile([128, H], bf16)
        nc.sync.dma_start(out=wj_s[:], in_=w1[0:E, :])
        wm_s = singles.tile([128, H], bf16)
        nc.sync.dma_start(out=wm_s[:], in_=w1[E:2 * E, :])
        w2_s = singles.tile([128, H], bf16)
        nc.sync.dma_start(out=w2_s[:], in_=w2[:])
        w3_s = singles.tile([128, 1], bf16)
        nc.sync.dma_start(out=w3_s[:], in_=w3[:])
        smat_s = singles.tile([KJV, NPAD], bf16)
        nc.sync.dma_start(out=smat_s[:], in_=smat[:])

        dcols = singles.tile([128, 2], bf16)
        nc.gpsimd.dma_start(out=dcols[:, 0:1],
                            in_=dvec[0:E].rearrange("(p o) -> p o", o=1))
        nc.gpsimd.dma_start(out=dcols[:, 1:2],
                            in_=dvec[E:2 * E].rearrange("(p o) -> p o", o=1))

        # all 16 indirect gathers first on the gpsimd queue, then bf16 casts
        grows_pool = ctx.enter_context(tc.tile_pool(name="growsp", bufs=NCHUNK))
        grows = [grows_pool.tile([128, E], f32, tag="grows", name=f"grows{c}")
                 for c in range(NCHUNK)]
        mrows_pool = ctx.enter_context(tc.tile_pool(name="mrowsp", bufs=NCHUNK))
        mrows = [mrows_pool.tile([128, E], f32, tag="mrows", name=f"mrows{c}")
                 for c in range(NCHUNK)]
        for c in range(NCHUNK):
            nc.gpsimd.indirect_dma_start(
                out=grows[c][:], out_offset=None, in_=ops[:],
                in_offset=bass.IndirectOffsetOnAxis(ap=idx_s[:, c:c + 1], axis=0),
            )
            nc.sync.dma_start(out=mrows[c][0:2 * N_MA, :],
                              in_=ma[2 * c * N_MA:(2 * c + 2) * N_MA, :])

        # small partition-strided loads on the scalar HWDGE ring
        b2_s = singles.tile([128, 1], f32)
        nc.scalar.dma_start(out=b2_s[:], in_=b2v[:].rearrange("(p o) -> p o", o=1))
        ident = singles.tile([128, 128], f32)
        from concourse.masks import make_identity
        make_identity(nc, ident[:])

        # zdb1: row 0 = dummy@W1 (filled below), row 1 = b1
        zdb1 = singles.tile([2, 128], bf16)
        nc.scalar.dma_start(out=zdb1[1:2, :],
                            in_=b1v[:].rearrange("(o e) -> o e", o=1))

        # xbar transposes (bf16, 16-bit path), off the compute engines
        jt_pool = ctx.enter_context(tc.tile_pool(name="jt", bufs=4))
        mt_pool = ctx.enter_context(tc.tile_pool(name="mt", bufs=4))

        # per-chunk JV tiles (2 batches side by side).  Rows 50:64 hold junk
        # projections of the finite pad gather rows; smat rows there are 0.
        jv_pool = ctx.enter_context(tc.tile_pool(name="jvp", bufs=NCHUNK))
        jvp = [jv_pool.tile([128, 2 * 128], bf16, tag="jv", name=f"jv{c}")
               for c in range(NCHUNK)]

        setup_ps = ctx.enter_context(tc.tile_pool(name="sps", bufs=2, space="PSUM"))
        h1_ps = ctx.enter_context(tc.tile_pool(name="h1ps", bufs=3, space="PSUM"))
        h2_ps = ctx.enter_context(tc.tile_pool(name="h2ps", bufs=2, space="PSUM"))
        lg_ps = ctx.enter_context(tc.tile_pool(name="lgps", bufs=1, space="PSUM"))
        a_pool = ctx.enter_context(tc.tile_pool(name="ap", bufs=4))
        h2s_pool = ctx.enter_context(tc.tile_pool(name="h2s", bufs=4))
        st_pool = ctx.enter_context(tc.tile_pool(name="st", bufs=3))

        # PE warm-up during the initial DMA window: junk matmuls (HAM)
        warm = singles.tile([128, 512], bf16)
        nc.vector.memset(warm[:].bitcast(mybir.dt.uint16), 0)
        for _ in range(8):
            wp = setup_ps.tile([128, 512], f32, tag="sps", name="wp")
            nc.tensor.matmul(out=wp[:], lhsT=warm[:, 0:128], rhs=warm[:],
                             start=True, stop=True)

        # dummy @ W1 (once): [1,128] = dcols[:,0].T @ Wj + dcols[:,1].T @ Wm
        pd = setup_ps.tile([1, 128], f32, tag="sps", name="pd")
        nc.tensor.matmul(out=pd[:], lhsT=dcols[:, 0:1], rhs=wj_s[:],
                         start=True, stop=False)
        nc.tensor.matmul(out=pd[:], lhsT=dcols[:, 1:2], rhs=wm_s[:],
                         start=False, stop=True)
        nc.vector.tensor_copy(out=zdb1[0:1, :], in_=pd[:])

        for c in range(NCHUNK):
            tpj = setup_ps.tile([128, 128], f32, tag="sps")
            nc.tensor.transpose(out=tpj[:], in_=grows[c][:], identity=ident[:])
            jTc = jt_pool.tile([128, 128], bf16, tag="jt")
            nc.scalar.copy(out=jTc[:], in_=tpj[:])
            tpm = setup_ps.tile([128, 128], f32, tag="sps")
            nc.tensor.transpose(out=tpm[:], in_=mrows[c][:], identity=ident[:])
            mTc = mt_pool.tile([128, 128], bf16, tag="mt")
            nc.vector.tensor_copy(out=mTc[:], in_=tpm[:])

            bb = (2 * c, 2 * c + 1)
            # ---- projections for both batches into jvp[c] ----
            pj = setup_ps.tile([128, 2 * 128], f32, tag="sps", name=f"pj{c}")
            for sub in range(2):
                nc.tensor.matmul(out=pj[0:PB, 128 * sub:128 * (sub + 1)],
                                 lhsT=jTc[:, sub * PB: (sub + 1) * PB],
                                 rhs=wj_s[:], start=True, stop=True)
                nc.tensor.matmul(out=pj[R_V0:R_V0 + N_MA, 128 * sub:128 * (sub + 1)],
                                 lhsT=mTc[:, sub * N_MA: (sub + 1) * N_MA],
                                 rhs=wm_s[:], start=True, stop=True)
            nc.scalar.copy(out=jvp[c][0:PB, :], in_=pj[0:PB, :])
            nc.vector.tensor_copy(out=jvp[c][R_V0:R_V0 + N_MA, :],
                                  in_=pj[R_V0:R_V0 + N_MA, :])
            nc.scalar.dma_start(
                out=jvp[c][R_ZD:R_B1 + 1, :].rearrange("p (r e) -> p r e", r=2),
                in_=zdb1[:].rearrange("p (o e) -> p o e", o=1).to_broadcast([2, 2, 128]))

            # ---- main pipelines, two batches interleaved ----
            lgps = [lg_ps.tile([128, 512], f32, tag="lg", name=f"lg{b}")
                    for b in bb]
            for ci in range(NCH):
                for sub in range(2):
                    b = bb[sub]
                    h1p = h1_ps.tile([128, 512], f32, tag="h1p")
                    nc.tensor.matmul(
                        out=h1p[:],
                        lhsT=jvp[c][0:KJV, 128 * sub:128 * (sub + 1)],
                        rhs=smat_s[:, 512 * ci:512 * (ci + 1)],
                        start=True, stop=True)
                    A = a_pool.tile([128, 512], bf16, tag="A")
                    if ci == 3 and (c + sub) % 2 == 0:
                        nc.scalar.activation(out=A[:], in_=h1p[:], func=Relu)
                    else:
                        nc.vector.tensor_scalar_max(out=A[:], in0=h1p[:],
                                                    scalar1=0.0)
                    h2p = h2_ps.tile([128, 512], f32, tag="h2p")
                    nc.tensor.matmul(out=h2p[:], lhsT=w2_s[:], rhs=A[:],
                                     start=True, stop=True)
                    H2 = h2s_pool.tile([128, 512], bf16, tag="H2")
                    nc.scalar.activation(out=H2[:], in_=h2p[:],
                                         func=Relu, bias=b2_s[:, 0:1])
                    # logits chunk -> psum partition 32*ci
                    nc.tensor.matmul(out=lgps[sub][32 * ci:32 * ci + 1, :],
                                     lhsT=w3_s[:], rhs=H2[:],
                                     start=True, stop=True,
                                     tile_position=(0, 32 * ci))
            for sub in range(2):
                b = bb[sub]
                # one wide copy (only rows 0/32/64/96 carry data)
                stg = st_pool.tile([128, 512], f32, tag="st")
                if b % 2 == 0:
                    nc.scalar.copy(out=stg[0:97, :], in_=lgps[sub][0:97, :])
                else:
                    nc.vector.tensor_copy(out=stg[0:97, :], in_=lgps[sub][0:97, :])
                stg4 = stg[:].rearrange("(a b) f -> a b f", b=32)[:, 0:1, :]
                nc.sync.dma_start(
                    out=out[b:b + 1, :].rearrange("o (a f) -> o a f", a=4),
                    in_=stg4)

    nc.finalize()
    return nc


_CACHE: dict = {}


def _get_module() -> bass.Bass:
    if "nc" not in _CACHE:
        _CACHE["nc"] = _build_module()
    return _CACHE["nc"]


def _make_in_maps(inputs):
    import ml_dtypes
    bf = ml_dtypes.bfloat16

    ops_emb = np.ascontiguousarray(np.asarray(inputs["ops_emb"], dtype=np.float32))
    ma_emb = np.ascontiguousarray(np.asarray(inputs["ma_emb"], dtype=np.float32))
    next_op = np.asarray(inputs["next_op"])
    dummy = np.asarray(inputs["dummy"], dtype=np.float32)
    W1 = np.ascontiguousarray(np.asarray(inputs["W1"], dtype=np.float32).astype(bf))
    b1 = np.asarray(inputs["b1"], dtype=np.float32).astype(bf)
    W2 = np.ascontiguousarray(np.asarray(inputs["W2"], dtype=np.float32).astype(bf))
    b2 = np.ascontiguousarray(np.asarray(inputs["b2"], dtype=np.float32))
    W3 = np.ascontiguousarray(np.asarray(inputs["W3"], dtype=np.float32).astype(bf))
    smat = _build_smat().astype(bf)

    in_maps = []
    for core in range(NCORES):
        bsl = slice(core * BPC, (core + 1) * BPC)
        no = np.asarray(next_op[bsl], dtype=np.int64)          # [BPC, 50]
        gidx = np.zeros((BPC, PB), np.int64)
        gidx[:, :N_JOBS] = no + (np.arange(BPC, dtype=np.int64)[:, None] * N_OPS)
        idx2d = np.ascontiguousarray(
            gidx.reshape(NCHUNK, 128).T.astype(np.int32))      # [128, NCHUNK]
        in_maps.append({
            "ops": ops_emb[bsl].reshape(BPC * N_OPS, E),
            "ma": ma_emb[bsl].reshape(BPC * N_MA, E),
            "idx": idx2d,
            "smat": smat,
            "w1": W1, "w2": W2, "w3": W3,
            "b1v": b1, "b2v": b2, "dvec": dummy,
        })
    return in_maps


def _run(inputs, trace=False, **kw):
    action_mask = np.asarray(inputs["action_mask"])
    b3 = np.asarray(inputs["b3"], dtype=np.float32)
    nc = _get_module()
    in_maps = _make_in_maps(inputs)
    res = run_bass_kernel_spmd(nc, in_maps, core_ids=list(range(NCORES)),
                               trace=trace, **kw)
    logits = np.concatenate([r["out"][:, :NPAIR] for r in res.results], axis=0)
    logits = (logits + b3.reshape(-1)[0]).astype(np.float32)
    return (logits, action_mask), res


def kernel(**inputs):
    out, _ = _run(inputs)
    return out
